# revision 1
# baseline (speedup 1.0000x reference)
"""Trainium2 Bass kernel for nn_AttentionalGNN (8-core SPMD, sequence-sharded).

Strategy:
  - Shard the N=1024 node axis across 8 cores (128 queries each). All of
    attention/merge/MLP/LayerNorm is local per position; only k/v need the full
    sequence, shared via one fused AllGather per layer.
  - Heads are made contiguous by permuting projection output channels host-side
    (orig channel d = 4a+b -> row b*64+a), with the merge weight columns
    permuted to match.
  - The final output only uses the layer-5 scores of prop(0,5,d1,d0), reduced
    over heads and queries: out[m] = (1/32) * (Wq5 @ mean_n d1)^T (Wk5 @ d0)[:,m].
    So layer 5 collapses to two projections + a tiny cross-core reduce.
"""
import numpy as np

import concourse.bass as bass
import concourse.bacc as bacc
import concourse.mybir as mybir
import concourse.tile as tile
from concourse.bass_utils import run_bass_kernel_spmd

D, N, H, DH = 256, 1024, 4, 64
NC = 8
CH = N // NC  # 128 positions per core
F32 = mybir.dt.float32
BF16 = mybir.dt.bfloat16
AF = mybir.ActivationFunctionType

PROPS_SELF = [(0, 0, 0), (0, 1, 1), (1, 2, 2), (2, 3, 3)]
PROPS_CROSS = [(0, 0, 1), (0, 1, 0), (1, 2, 1), (1, 1, 2), (2, 0, 3), (2, 3, 0)]
NAMES5 = ['self', 'cross', 'self', 'cross', 'self']
PERM = np.array([4 * (r % 64) + r // 64 for r in range(256)])

_cache = {}


def _props(i):
    return PROPS_CROSS if NAMES5[i] == 'cross' else PROPS_SELF


def build_kernel(trace_scopes=False, n_layers=5, stages="ABC", blvl=9, reps=1, zb=True):
    nc = bacc.Bacc("TRN2", target_bir_lowering=False, debug=False, num_devices=NC)

    # ---- I/O ----
    xc = nc.dram_tensor("xc", [4, 2, 128, CH], F32, kind="ExternalInput")
    wqkvT = nc.dram_tensor("wqkvT", [5, 3, 3, 256, 256], BF16, kind="ExternalInput")
    mergeT = nc.dram_tensor("mergeT", [5, 3, 256, 256], BF16, kind="ExternalInput")
    w1T = nc.dram_tensor("w1T", [5, 3, 512, 512], BF16, kind="ExternalInput")
    w2T = nc.dram_tensor("w2T", [5, 3, 512, 256], BF16, kind="ExternalInput")
    pbq = nc.dram_tensor("pbq", [5, 3, 256], F32, kind="ExternalInput")
    pbk = nc.dram_tensor("pbk", [5, 3, 256], F32, kind="ExternalInput")
    pbv = nc.dram_tensor("pbv", [5, 3, 256], F32, kind="ExternalInput")
    mbb = nc.dram_tensor("mbb", [5, 3, 256], F32, kind="ExternalInput")
    b1b = nc.dram_tensor("b1b", [5, 3, 512], F32, kind="ExternalInput")
    b2b = nc.dram_tensor("b2b", [5, 3, 256], F32, kind="ExternalInput")
    lng = nc.dram_tensor("lng", [5, 4, 256], F32, kind="ExternalInput")
    lnb = nc.dram_tensor("lnb", [5, 4, 256], F32, kind="ExternalInput")
    w5T = nc.dram_tensor("w5T", [2, 256, 256], F32, kind="ExternalInput")  # [qT,kT]
    pb5 = nc.dram_tensor("pb5", [2, 256], F32, kind="ExternalInput")
    sel = nc.dram_tensor("sel", [2, 4, 128], F32, kind="ExternalInput")
    out_d = nc.dram_tensor("out", [1, CH], F32, kind="ExternalOutput")

    # per-layer collective buffers (compile-time static)
    agins, agouts = [], []
    for i in range(5):
        np_ = len(_props(i))
        agins.append(nc.dram_tensor(f"agin{i}", [np_, 4, 128, 128], BF16))
        agouts.append(nc.dram_tensor(f"agout{i}", [NC, np_, 4, 128, 128], BF16,
                                     addr_space="Shared"))
    ag2in = nc.dram_tensor("ag2in", [2, 128, 1], F32)
    ag2out = nc.dram_tensor("ag2out", [NC, 2, 128, 1], F32, addr_space="Shared")

    # persistent SBUF state
    dst = nc.alloc_sbuf_tensor("dstate", [128, 4, 2, CH], F32)   # d[t] ctile c
    dlt = nc.alloc_sbuf_tensor("delta", [128, 4, 2, CH], F32)
    dstb = nc.alloc_sbuf_tensor("dstateb", [128, 4, 2, CH], BF16)

    rg = [list(range(NC))]

    from contextlib import ExitStack
    with ExitStack() as es:
        tc = es.enter_context(tile.TileContext(nc))
        cpool = es.enter_context(tc.tile_pool(name="const", bufs=1))
        wqp = es.enter_context(tc.tile_pool(name="wq", bufs=2))
        wkp = es.enter_context(tc.tile_pool(name="wk", bufs=2))
        wvp = es.enter_context(tc.tile_pool(name="wv", bufs=2))
        mgp = es.enter_context(tc.tile_pool(name="mg", bufs=2))
        w1p = es.enter_context(tc.tile_pool(name="w1", bufs=2))
        w2p = es.enter_context(tc.tile_pool(name="w2", bufs=2))
        bp = es.enter_context(tc.tile_pool(name="bias", bufs=3))
        khp = es.enter_context(tc.tile_pool(name="kh", bufs=2))
        vtp = es.enter_context(tc.tile_pool(name="vt", bufs=2))
        ap_ = es.enter_context(tc.tile_pool(name="act", bufs=3))
        ep = es.enter_context(tc.tile_pool(name="exps", bufs=10))
        sp_ = es.enter_context(tc.tile_pool(name="small", bufs=4))
        ps = es.enter_context(tc.tile_pool(name="ps", bufs=3, space="PSUM"))
        scp = es.enter_context(tc.tile_pool(name="sc", bufs=2, space="PSUM"))
        avp = es.enter_context(tc.tile_pool(name="av", bufs=1, space="PSUM"))
        zp = es.enter_context(tc.tile_pool(name="zp", bufs=1, space="PSUM"))
        if True:
            ones_c = cpool.tile([128, 1], F32, name="tl", tag="ones_c")
            nc.gpsimd.memset(ones_c[:], 1.0)
            ones_r = cpool.tile([1, 128], F32, name="tl", tag="ones_r")
            nc.gpsimd.memset(ones_r[:], 1.0)
            eps_c = cpool.tile([1, 1], F32, name="tl", tag="eps_c")
            nc.gpsimd.memset(eps_c[:], 1e-5)
            ones64 = cpool.tile([128, 64], F32, name="tl", tag="ones64")
            nc.gpsimd.memset(ones64[:], 1.0)
            ones64b = cpool.tile([128, 64], BF16, name="tl", tag="ones64b")
            nc.gpsimd.memset(ones64b[:], 1.0)
            qpad = [cpool.tile([128, 256], BF16, name="tl", tag=f"qpad{c}") for c in range(2)]
            for c in range(2):
                nc.gpsimd.memset(qpad[c][:], 0.0)

            # load descriptor chunks into d-state
            for t in range(4):
                for c in range(2):
                    nc.sync.dma_start(dst[:, t, c, :], xc[t, c])
                    nc.vector.tensor_copy(dstb[:, t, c, :], dst[:, t, c, :])

            def load_w(pool, src_ap, n_tiles, cols, tag):
                ts_ = [pool.tile([128, cols], BF16, name="tl", tag=f"{tag}{k}") for k in range(n_tiles)]
                for k in range(n_tiles):
                    nc.sync.dma_start(ts_[k][:], src_ap[k * 128:(k + 1) * 128, :])
                return ts_

            def load_bias(src_ap, n, tag):
                t_ = bp.tile([128, n // 128], F32, name="tl", tag=tag)
                nc.sync.dma_start(t_[:], src_ap.rearrange("(a p) -> p a", p=128))
                return t_

            for i in [li for _r in range(reps) for li in range(n_layers)]:
                props = _props(i)
                brs = sorted(set(p[0] for p in props))

                # ---- stage A: produce local k/v chunks for every prop, then AllGather
                for br in (brs if "A" in stages else []):
                    wk_t = load_w(wkp, wqkvT[i, br, 1], 2, 256, "wk")
                    wv_t = load_w(wvp, wqkvT[i, br, 2], 2, 256, "wv")
                    bk_t = load_bias(pbk[i, br], 256, "bk")
                    # broadcast v-bias along partitions: [128,256] = ones_c x bv_row
                    bvr = sp_.tile([1, 256], F32, name="tl", tag="bvr")
                    nc.sync.dma_start(bvr[:], pbv[i, br][None, :])
                    bv_ps = ps.tile([128, 256], F32, name="tl", tag="ps")
                    nc.tensor.matmul(bv_ps[:], ones_r[:], bvr[:], start=True, stop=True)
                    bv_b = sp_.tile([128, 256], F32, name="tl", tag="bvb")
                    nc.vector.tensor_copy(bv_b[:], bv_ps[:])
                    for pi, (br_, xi, si) in enumerate(props):
                        if br_ != br:
                            continue
                        # k chunk: [256, CH]
                        for c in range(2):
                            kc_ps = ps.tile([128, CH], F32, name="tl", tag="ps")
                            for cc in range(2):
                                nc.tensor.matmul(kc_ps[:], wk_t[cc][:, c * 128:(c + 1) * 128],
                                                 dstb[:, si, cc, :], start=(cc == 0), stop=(cc == 1))
                            kc_sb = ap_.tile([128, CH], BF16, name="tl", tag="kcsb")
                            if zb:
                                nc.vector.tensor_copy(kc_sb[:], kc_ps[:])
                            else:
                                nc.scalar.activation(kc_sb[:], kc_ps[:], AF.Identity, bias=bk_t[:, c:c + 1])
                            nc.sync.dma_start(agins[i][pi, c], kc_sb[:])
                        # vT chunk: [CH, 256]
                        vt_ps = ps.tile([128, 256], F32, name="tl", tag="ps")
                        for cc in range(2):
                            nc.tensor.matmul(vt_ps[:], dstb[:, si, cc, :], wv_t[cc][:],
                                             start=(cc == 0), stop=(cc == 1))
                        vt_sb = ap_.tile([128, 256], BF16, name="tl", tag="vtsb")
                        if zb:
                            nc.vector.tensor_copy(vt_sb[:], vt_ps[:])
                        else:
                            nc.vector.tensor_add(vt_sb[:], vt_ps[:], bv_b[:])
                        for c in range(2):
                            nc.sync.dma_start(agins[i][pi, 2 + c], vt_sb[:, c * 128:(c + 1) * 128])

                if "A" in stages:
                    nc.gpsimd.collective_compute(
                        "AllGather", mybir.AluOpType.bypass, replica_groups=rg,
                        ins=[agins[i].ap().opt()], outs=[agouts[i].ap().opt()])

                # ---- stage B: per prop attention + merge + MLP on local queries
                first_delta = {}
                for t in range(4):
                    first_delta[t] = True
                cur_br = None
                wq_t = mg_t = w1_t = w2_t = None
                bq_t = mb_t = b1_t = b2_t = None
                for pi, (br, xi, si) in (list(enumerate(props)) if "B" in stages else []):
                    if br != cur_br:
                        cur_br = br
                        wq_t = load_w(wqp, wqkvT[i, br, 0], 2, 256, "wq")
                        mg_t = load_w(mgp, mergeT[i, br], 2, 256, "mg")
                        w1_t = load_w(w1p, w1T[i, br], 4, 512, "w1")
                        w2_t = load_w(w2p, w2T[i, br], 4, 256, "w2")
                        bq_t = load_bias(pbq[i, br], 256, "bq")
                        mb_t = load_bias(mbb[i, br], 256, "mb")
                        b1_t = load_bias(b1b[i, br], 512, "b1")
                        b2_t = load_bias(b2b[i, br], 256, "b2")

                    # gathered k: [256, N] as 2 ctiles; vT: 8 mtiles [128,256]
                    kh_t = [khp.tile([128, N], BF16, name="tl", tag=f"kh{c}") for c in range(2)]
                    ag_r = agouts[i].ap().rearrange("r q c p f -> q c p r f")
                    for c in range(2):
                        nc.sync.dma_start(kh_t[c][:].rearrange("p (r f) -> p r f", r=NC),
                                          ag_r[pi, c])
                    vt_t = [vtp.tile([128, 256], BF16, name="tl", tag=f"vt{m}") for m in range(NC)]
                    ag_v = agouts[i].ap().rearrange("r q c p f -> r q p c f")
                    for m in range(NC):
                        nc.sync.dma_start(vt_t[m][:].rearrange("p (c f) -> p c f", c=2),
                                          ag_v[m, pi, :, 2:4, :])

                    # q: [256, CH] scaled by 1/8, evicted into block-diagonal qpad
                    for c in range(2):
                        q_ps = ps.tile([128, CH], F32, name="tl", tag="ps")
                        for cc in range(2):
                            nc.tensor.matmul(q_ps[:], wq_t[cc][:, c * 128:(c + 1) * 128],
                                             dstb[:, xi, cc, :], start=(cc == 0), stop=(cc == 1))
                        if zb:
                            nc.vector.tensor_copy(qpad[c][0:64, 0:CH], q_ps[0:64, :])
                            nc.vector.tensor_copy(qpad[c][64:128, CH:2 * CH], q_ps[64:128, :])
                        else:
                            nc.scalar.activation(qpad[c][0:64, 0:CH], q_ps[0:64, :], AF.Identity,
                                                 bias=bq_t[0:64, c:c + 1])
                            nc.scalar.activation(qpad[c][64:128, CH:2 * CH], q_ps[64:128, :], AF.Identity,
                                                 bias=bq_t[64:128, c:c + 1])

                    # attention: scores^T via block-diag qpad (K=128), exp, Z, A@V
                    if blvl < 2: continue
                    z_acc = ep.tile([128, 4 * CH], BF16, name="tl", tag="zacc")
                    av_ps = [avp.tile([128, CH], F32, name="tl", tag=f"av{c}") for c in range(2)]
                    for m in range(NC):
                        sc_ps = scp.tile([128, 4 * CH], F32, name="tl", tag="sc")
                        for c in range(2):
                            nc.tensor.matmul(sc_ps[:, (2 * c) * CH:(2 * c + 2) * CH],
                                             kh_t[c][:, m * 128:(m + 1) * 128],
                                             qpad[c][:], start=True, stop=True)
                        e_sb = ep.tile([128, 4 * CH], BF16, name="tl", tag="exps")
                        nc.scalar.activation(e_sb[:], sc_ps[:], AF.Exp)
                        if blvl >= 3:
                            if m == 0:
                                nc.vector.tensor_copy(z_acc[:], e_sb[:])
                            else:
                                nc.vector.tensor_add(z_acc[:], z_acc[:], e_sb[:])
                        for h in (range(H) if blvl >= 4 else []):
                            c, o = h // 2, 64 * (h % 2)
                            nc.tensor.matmul(av_ps[c][o:o + 64, :],
                                             vt_t[m][:, 64 * h:64 * h + 64],
                                             e_sb[:, h * CH:(h + 1) * CH],
                                             start=(m == 0), stop=(m == NC - 1),
                                             tile_position=(0, o))
                    z_ps = zp.tile([64, 4 * CH], F32, name="tl", tag="z")
                    if blvl >= 3:
                        nc.tensor.matmul(z_ps[:], ones64b[:], z_acc[:], start=True, stop=True)
                    # normalize: r_row[0, h*CH:] = 1/Z_h ; Bc rows 64h..: broadcast ; attn = av * Bc
                    if blvl < 5: continue
                    r_row = sp_.tile([1, 4 * CH], F32, name="tl", tag="rz")
                    nc.vector.tensor_copy(r_row[:], z_ps[0:1, :])
                    nc.vector.reciprocal(r_row[:], r_row[:])
                    attn_sb = [ap_.tile([128, CH], BF16, name="tl", tag=f"at{c}") for c in range(2)]
                    for c in range(2):
                        b_ps = ps.tile([128, CH], F32, name="tl", tag="ps")
                        for hh in range(2):
                            h = 2 * c + hh
                            nc.tensor.matmul(b_ps[64 * hh:64 * hh + 64, :], ones_r[:, 0:64],
                                             r_row[:, h * CH:(h + 1) * CH],
                                             start=True, stop=True, tile_position=(0, 64 * hh))
                        b_sb = ap_.tile([128, CH], F32, name="tl", tag="bcsb")
                        nc.vector.tensor_copy(b_sb[:], b_ps[:])
                        nc.vector.tensor_mul(attn_sb[c][:], av_ps[c][:], b_sb[:])

                    # merge
                    if blvl < 6: continue
                    msg_sb = [ap_.tile([128, CH], BF16, name="tl", tag=f"ms{c}") for c in range(2)]
                    for c in range(2):
                        m_ps = ps.tile([128, CH], F32, name="tl", tag="ps")
                        for cc in range(2):
                            nc.tensor.matmul(m_ps[:], mg_t[cc][:, c * 128:(c + 1) * 128],
                                             attn_sb[cc][:], start=(cc == 0), stop=(cc == 1))
                        if zb:
                            nc.vector.tensor_copy(msg_sb[c][:], m_ps[:])
                        else:
                            nc.scalar.activation(msg_sb[c][:], m_ps[:], AF.Identity, bias=mb_t[:, c:c + 1])

                    # mlp1 (relu) on concat([x, msg])
                    if blvl < 7: continue
                    h_in = [dstb[:, xi, 0, :], dstb[:, xi, 1, :], msg_sb[0][:], msg_sb[1][:]]
                    h1_sb = [ap_.tile([128, CH], BF16, name="tl", tag=f"h1{c}") for c in range(4)]
                    for c in range(4):
                        h_ps = ps.tile([128, CH], F32, name="tl", tag="ps")
                        for cc in range(4):
                            nc.tensor.matmul(h_ps[:], w1_t[cc][:, c * 128:(c + 1) * 128],
                                             h_in[cc], start=(cc == 0), stop=(cc == 3))
                        if zb:
                            nc.vector.tensor_relu(h1_sb[c][:], h_ps[:])
                        else:
                            nc.scalar.activation(h1_sb[c][:], h_ps[:], AF.Relu, bias=b1_t[:, c:c + 1])

                    # mlp2 -> delta accumulation
                    if blvl < 8: continue
                    for c in range(2):
                        d_ps = ps.tile([128, CH], F32, name="tl", tag="ps")
                        for cc in range(4):
                            nc.tensor.matmul(d_ps[:], w2_t[cc][:, c * 128:(c + 1) * 128],
                                             h1_sb[cc][:], start=(cc == 0), stop=(cc == 3))
                        if first_delta[xi]:
                            if zb:
                                nc.vector.tensor_copy(dlt[:, xi, c, :], d_ps[:])
                            else:
                                nc.scalar.activation(dlt[:, xi, c, :], d_ps[:], AF.Identity,
                                                     bias=b2_t[:, c:c + 1])
                        else:
                            if zb:
                                nc.vector.tensor_add(dlt[:, xi, c, :], dlt[:, xi, c, :], d_ps[:])
                            else:
                                tmp = ap_.tile([128, CH], F32, name="tl", tag="dtmp")
                                nc.scalar.activation(tmp[:], d_ps[:], AF.Identity, bias=b2_t[:, c:c + 1])
                                nc.vector.tensor_add(dlt[:, xi, c, :], dlt[:, xi, c, :], tmp[:])
                    first_delta[xi] = False

                # ---- stage C: residual + LayerNorm per tensor
                g_row = sp_.tile([1, 256], F32, name="tl", tag="grow")
                b_col = bp.tile([128, 2], F32, name="tl", tag="lnb")
                for t in (range(4) if "C" in stages else []):
                    nc.sync.dma_start(g_row[:], lng[i, t][None, :])
                    nc.sync.dma_start(b_col[:], lnb[i, t].rearrange("(a p) -> p a", p=128))
                    xnq = [ap_.tile([128, 2 * CH], F32, name="tl", tag=f"xn{c}") for c in range(2)]
                    s_ps = ps.tile([128, 2 * CH], F32, name="tl", tag="ps")
                    for c in range(2):
                        nc.vector.tensor_add(xnq[c][:, 0:CH], dst[:, t, c, :], dlt[:, t, c, :])
                        nc.vector.tensor_mul(xnq[c][:, CH:2 * CH], xnq[c][:, 0:CH], xnq[c][:, 0:CH])
                        nc.tensor.matmul(s_ps[0:64, :], ones64[:], xnq[c][:],
                                         start=(c == 0), stop=(c == 1))
                    xn = [xnq[c][:, 0:CH] for c in range(2)]
                    mu = sp_.tile([1, CH], F32, name="tl", tag="mu")
                    msq = sp_.tile([1, CH], F32, name="tl", tag="msq")
                    nc.vector.tensor_scalar_mul(mu[:], s_ps[0:1, 0:CH], 1.0 / 256)
                    nc.vector.tensor_scalar_mul(msq[:], s_ps[0:1, CH:2 * CH], 1.0 / 256)
                    var = sp_.tile([1, CH], F32, name="tl", tag="var")
                    nc.vector.tensor_mul(var[:], mu[:], mu[:])
                    nc.vector.tensor_sub(var[:], msq[:], var[:])
                    sd = sp_.tile([1, CH], F32, name="tl", tag="sd")
                    nc.scalar.activation(sd[:], var[:], AF.Sqrt, bias=eps_c[:])
                    rs = sp_.tile([1, CH], F32, name="tl", tag="rs")
                    nc.vector.reciprocal(rs[:], sd[:])
                    mu_ps = ps.tile([128, CH], F32, name="tl", tag="ps")
                    nc.tensor.matmul(mu_ps[:], ones_r[:], mu[:], start=True, stop=True)
                    for c in range(2):
                        b2_ps = ps.tile([128, CH], F32, name="tl", tag="ps")
                        nc.tensor.matmul(b2_ps[:], g_row[:, c * 128:(c + 1) * 128], rs[:],
                                         start=True, stop=True)
                        t1 = ap_.tile([128, CH], F32, name="tl", tag="t1")
                        nc.vector.tensor_sub(t1[:], xn[c], mu_ps[:])
                        nc.vector.tensor_mul(t1[:], t1[:], b2_ps[:])
                        nc.vector.tensor_scalar_add(dst[:, t, c, :], t1[:], b_col[:, c:c + 1])
                        nc.vector.tensor_copy(dstb[:, t, c, :], dst[:, t, c, :])

            # ---- epilogue: out[m] = (1/32) qvec^T kmat[:, m]
            s1 = sp_.tile([128, 2], F32, name="tl", tag="s1")
            for c in range(2):
                nc.vector.reduce_sum(s1[:, c:c + 1], dst[:, 1, c, :], axis=mybir.AxisListType.X)
                nc.sync.dma_start(ag2in[c], s1[:, c:c + 1])
            nc.gpsimd.collective_compute(
                "AllGather", mybir.AluOpType.bypass, replica_groups=rg,
                ins=[ag2in.ap().opt()], outs=[ag2out.ap().opt()])
            d1b = sp_.tile([128, 2], F32, name="tl", tag="d1b")
            gath = sp_.tile([128, NC], F32, name="tl", tag="gath")
            for c in range(2):
                nc.sync.dma_start(gath[:], ag2out.ap().rearrange("r c p o -> c p (r o)")[c])
                nc.vector.reduce_sum(d1b[:, c:c + 1], gath[:], axis=mybir.AxisListType.X)

            wq5 = [cpool.tile([128, 256], F32, name="tl", tag=f"wq5{k}") for k in range(2)]
            wk5 = [cpool.tile([128, 256], F32, name="tl", tag=f"wk5{k}") for k in range(2)]
            for k in range(2):
                nc.sync.dma_start(wq5[k][:], w5T[0, k * 128:(k + 1) * 128, :])
                nc.sync.dma_start(wk5[k][:], w5T[1, k * 128:(k + 1) * 128, :])
            b5 = bp.tile([128, 4], F32, name="tl", tag="b5")
            nc.sync.dma_start(b5[:], pb5.rearrange("t (a p) -> p (t a)", p=128))
            qv = sp_.tile([128, 2], F32, name="tl", tag="qv")
            for c in range(2):
                q_ps = ps.tile([128, CH], F32, name="tl", tag="ps")
                for cc in range(2):
                    nc.tensor.matmul(q_ps[:, 0:1], wq5[cc][:, c * 128:(c + 1) * 128],
                                     d1b[:, cc:cc + 1], start=(cc == 0), stop=(cc == 1))
                nc.scalar.activation(qv[:, c:c + 1], q_ps[:, 0:1], AF.Identity,
                                     bias=b5[:, c:c + 1], scale=1.0 / N)
            km = [ap_.tile([128, CH], F32, name="tl", tag=f"km{c}") for c in range(2)]
            for c in range(2):
                k_ps = ps.tile([128, CH], F32, name="tl", tag="ps")
                for cc in range(2):
                    nc.tensor.matmul(k_ps[:], wk5[cc][:, c * 128:(c + 1) * 128],
                                     dst[:, 0, cc, :], start=(cc == 0), stop=(cc == 1))
                nc.scalar.activation(km[c][:], k_ps[:], AF.Identity, bias=b5[:, 2 + c:3 + c])
            o_ps = ps.tile([128, CH], F32, name="tl", tag="ps")
            for c in range(2):
                nc.vector.tensor_scalar_mul(km[c][:], km[c][:], qv[:, c:c + 1])
                nc.tensor.matmul(o_ps[0:64, :], ones64[:], km[c][:],
                                 start=(c == 0), stop=(c == 1))
            o_sb = sp_.tile([1, CH], F32, name="tl", tag="osb")
            nc.scalar.activation(o_sb[:], o_ps[0:1, :], AF.Copy, scale=1.0 / 32)
            nc.sync.dma_start(out_d[:], o_sb[:])

    nc.compile()
    return nc


def prep_inputs(inputs):
    inp = {k: np.ascontiguousarray(np.asarray(v)) for k, v in inputs.items()}
    pw, pb = inp['proj_w'].astype(np.float32), inp['proj_b'].astype(np.float32)
    mw, mb = inp['merge_w'].astype(np.float32), inp['merge_b'].astype(np.float32)
    w1, b1 = inp['mlp_w1'].astype(np.float32), inp['mlp_b1'].astype(np.float32)
    w2, b2 = inp['mlp_w2'].astype(np.float32), inp['mlp_b2'].astype(np.float32)
    ng, nb = inp['norm_g'].astype(np.float32), inp['norm_b'].astype(np.float32)

    wqkvT = np.empty((5, 3, 3, 256, 256), np.float32)
    mergeT = np.empty((5, 3, 256, 256), np.float32)
    w1T = np.empty((5, 3, 512, 512), np.float32)
    w2T = np.empty((5, 3, 512, 256), np.float32)
    pbq = np.empty((5, 3, 256), np.float32)
    pbk = np.empty((5, 3, 256), np.float32)
    pbv = np.empty((5, 3, 256), np.float32)
    for i in range(5):
        for br in range(3):
            for j in range(3):
                wqkvT[i, br, j] = pw[br, i, j][PERM].T
            wqkvT[i, br, 0] *= 0.125
            pbq[i, br] = pb[br, i, 0][PERM] * 0.125
            pbk[i, br] = pb[br, i, 1][PERM]
            pbv[i, br] = pb[br, i, 2][PERM]
            mergeT[i, br] = mw[br, i][:, PERM].T
            w1T[i, br] = w1[br, i].T
            w2T[i, br] = w2[br, i].T
    mbbv = np.transpose(mb[:, :5], (1, 0, 2)).astype(np.float32).copy()
    b1bv = np.transpose(b1[:, :5], (1, 0, 2)).astype(np.float32).copy()
    b2bv = np.transpose(b2[:, :5], (1, 0, 2)).astype(np.float32).copy()
    lngv = np.transpose(ng[:, :5], (1, 0, 2)).astype(np.float32).copy()
    lnbv = np.transpose(nb[:, :5], (1, 0, 2)).astype(np.float32).copy()
    w5T = np.stack([pw[0, 5, 0].T, pw[0, 5, 1].T]).astype(np.float32)
    pb5 = np.stack([pb[0, 5, 0], pb[0, 5, 1]]).astype(np.float32)
    sel = np.zeros((2, 4, 128), np.float32)
    for c in range(2):
        sel[c, 2 * c, 0:64] = 1.0
        sel[c, 2 * c + 1, 64:128] = 1.0

    desc = np.stack([inp[f'desc{t}'][0] for t in range(4)]).astype(np.float32)  # [4,256,N]
    bf = mybir.dt.np(mybir.dt.bfloat16)
    wqkvT = wqkvT.astype(bf); mergeT = mergeT.astype(bf)
    w1T = w1T.astype(bf); w2T = w2T.astype(bf)
    shared = dict(wqkvT=wqkvT, mergeT=mergeT, w1T=w1T, w2T=w2T, pbq=pbq, pbk=pbk,
                  pbv=pbv, mbb=mbbv, b1b=b1bv, b2b=b2bv, lng=lngv, lnb=lnbv,
                  w5T=w5T, pb5=pb5, sel=sel)
    in_maps = []
    for j in range(NC):
        xcj = desc[:, :, j * CH:(j + 1) * CH].reshape(4, 2, 128, CH)
        in_maps.append({"xc": np.ascontiguousarray(xcj), **shared})
    return in_maps


def kernel(**inputs):
    zb = all(not np.asarray(inputs[k]).any() for k in
             ("proj_b", "merge_b", "mlp_b1", "mlp_b2"))
    key = f"nc{zb}"
    if key not in _cache:
        _cache[key] = build_kernel(zb=zb)
    nc = _cache[key]
    in_maps = prep_inputs(inputs)
    res = run_bass_kernel_spmd(nc, in_maps, core_ids=list(range(NC)))
    out = np.concatenate([res.results[j]["out"][0] for j in range(NC)])
    mask = np.asarray(inputs["unreachable"]).any(axis=0)
    out = np.where(mask, np.float32(-1e9), out.astype(np.float32))
    return out



# revision 14
# speedup vs baseline: 1.2580x; 1.2580x over previous
"""Trainium2 Bass kernel for nn_AttentionalGNN (8-core SPMD, sequence-sharded) v2.

Strategy (vs v1 baseline):
  - N=1024 node axis sharded over 8 cores (CH=128 queries each); k/v computed
    locally per (branch, source) pair and shared via AllGather (split into two
    collectives per layer: br0 first so stage B can start while the rest fly).
  - All weights of one (layer, branch) packed host-side into single DRAM blobs
    -> one DMA each (was ~14).  k/v chunks packed [128, 512] per pair -> one
    write DMA + one 1MB gather-read DMA per prop (was ~10 small ones).
  - Dead code pruned: layer-3 d2/d3 updates, layer-4 d2/d3 props + LNs are
    never observable in the output (layer 5 only needs d0/d1).
  - Single ACT table set (natural_log_exp_and_others): LN rsqrt is computed as
    exp(-0.5*ln(var+eps)) instead of Sqrt, so the exp table never reloads.
  - Scores via two concurrent K=64 row-tiles (heads packed in partitions),
    exp over [128,1024] 2-bank PSUM tiles (amortizes ACT per-op overhead).
  - d state kept in bf16; residual+LN fused ops; per-prop DVE op count cut.
  - Final layer-5 score collapses to two projections + tiny AllGather
    (out[m] = (1/32)(Wq5 @ mean_n d1)^T (Wk5 @ d0)[:, m]).
"""
import numpy as np

import concourse.bass as bass
import concourse.bacc as bacc
import concourse.mybir as mybir
import concourse.tile as tile
from concourse.bass_utils import run_bass_kernel_spmd

D, N, H, DH = 256, 1024, 4, 64
NC = 8
CH = N // NC  # 128 positions per core
F32 = mybir.dt.float32
BF16 = mybir.dt.bfloat16
AF = mybir.ActivationFunctionType

# props per layer (br, xi, si), dead ones pruned (output only needs d0/d1
# after layer 4; layer-5 'cross' collapses into the epilogue).
LAYER_PROPS = [
    [(0, 0, 0), (0, 1, 1), (1, 2, 2), (2, 3, 3)],
    [(0, 0, 1), (0, 1, 0), (1, 2, 1), (1, 1, 2), (2, 0, 3), (2, 3, 0)],
    [(0, 0, 0), (0, 1, 1), (1, 2, 2), (2, 3, 3)],
    [(0, 0, 1), (0, 1, 0), (1, 1, 2), (2, 0, 3)],
    [(0, 0, 0), (0, 1, 1)],
]
LAYER_LNS = [[0, 1, 2, 3], [0, 1, 2, 3], [0, 1, 2, 3], [0, 1], [0, 1]]
PERM = np.array([4 * (r % 64) + r // 64 for r in range(256)])

_cache = {}
import os as _os
EXP_SPLIT = _os.environ.get("K_EXP_SPLIT", "0") == "1"
SC_ROWSPLIT = _os.environ.get("K_SC_ROWSPLIT", "0") == "1"


def build_kernel(trace_scopes=False, n_layers=5, stages="ABC", blvl=9, reps=1,
                 zb=True, ln_triv=True):
    nc = bacc.Bacc("TRN2", target_bir_lowering=False, debug=False, num_devices=NC)

    # ---- I/O ----
    xcb = nc.dram_tensor("xcb", [128, 4, 2, CH], BF16, kind="ExternalInput")
    wpackA = nc.dram_tensor("wpackA", [5, 3, 128, 1024], BF16, kind="ExternalInput")
    wpackB = nc.dram_tensor("wpackB", [5, 3, 128, 4096], BF16, kind="ExternalInput")
    w5T = nc.dram_tensor("w5T", [2, 256, 256], BF16, kind="ExternalInput")  # [qT,kT]
    pb5 = nc.dram_tensor("pb5", [2, 256], F32, kind="ExternalInput")
    if not zb:
        pbq = nc.dram_tensor("pbq", [5, 3, 256], F32, kind="ExternalInput")
        pbk = nc.dram_tensor("pbk", [5, 3, 256], F32, kind="ExternalInput")
        pbv = nc.dram_tensor("pbv", [5, 3, 256], F32, kind="ExternalInput")
        mbb = nc.dram_tensor("mbb", [5, 3, 256], F32, kind="ExternalInput")
        b1b = nc.dram_tensor("b1b", [5, 3, 512], F32, kind="ExternalInput")
        b2b = nc.dram_tensor("b2b", [5, 3, 256], F32, kind="ExternalInput")
    if not ln_triv:
        lng = nc.dram_tensor("lng", [5, 4, 256], F32, kind="ExternalInput")
        lnb = nc.dram_tensor("lnb", [5, 4, 256], F32, kind="ExternalInput")
    out_d = nc.dram_tensor("out", [1, CH], F32, kind="ExternalOutput")

    # per-(layer, group) collective buffers
    def grp_split(props):
        g0 = [j for j, p in enumerate(props) if p[0] == 0]
        g1 = [j for j, p in enumerate(props) if p[0] != 0]
        return g0, g1

    agin_t, agout_t = [], []
    for i in range(5):
        g0, g1 = grp_split(LAYER_PROPS[i])
        ins, outs = [], []
        for g, idxs in enumerate([g0, g1]):
            if idxs:
                ins.append(nc.dram_tensor(f"agin{i}_{g}", [len(idxs), 128, 512], BF16))
                outs.append(nc.dram_tensor(f"agout{i}_{g}", [NC, len(idxs), 128, 512],
                                           BF16, addr_space="Shared"))
            else:
                ins.append(None)
                outs.append(None)
        agin_t.append(ins)
        agout_t.append(outs)
    ag2in = nc.dram_tensor("ag2in", [2, 128, 1], F32)
    ag2out = nc.dram_tensor("ag2out", [NC, 2, 128, 1], F32, addr_space="Shared")

    # persistent SBUF state (bf16 d-state; f32 delta accumulator)
    dstb = nc.alloc_sbuf_tensor("dstateb", [128, 4, 2, CH], BF16)
    dlt = nc.alloc_sbuf_tensor("delta", [128, 4, 2 * CH], F32)

    rg = [list(range(NC))]

    from contextlib import ExitStack
    with ExitStack() as es:
        tc = es.enter_context(tile.TileContext(nc))
        cpool = es.enter_context(tc.tile_pool(name="const", bufs=1))
        wbp = es.enter_context(tc.tile_pool(name="wb", bufs=3))
        wap = es.enter_context(tc.tile_pool(name="wa", bufs=3))
        kvwp = es.enter_context(tc.tile_pool(name="kvw", bufs=4))
        kvgp = es.enter_context(tc.tile_pool(name="kvg", bufs=2))
        qhp = es.enter_context(tc.tile_pool(name="qh", bufs=6))
        ep = es.enter_context(tc.tile_pool(name="exps", bufs=6))
        zap = es.enter_context(tc.tile_pool(name="zacc", bufs=2))
        sp_ = es.enter_context(tc.tile_pool(name="small", bufs=4))
        abp = es.enter_context(tc.tile_pool(name="bcsb", bufs=2))
        atp = es.enter_context(tc.tile_pool(name="attn", bufs=2))
        msp = es.enter_context(tc.tile_pool(name="msg", bufs=2))
        h1p = es.enter_context(tc.tile_pool(name="h1", bufs=4))
        lnp = es.enter_context(tc.tile_pool(name="ln", bufs=2))
        tmpp = es.enter_context(tc.tile_pool(name="tmp", bufs=2))
        bp = es.enter_context(tc.tile_pool(name="bias", bufs=3))
        scp = es.enter_context(tc.tile_pool(name="sc", bufs=2, space="PSUM"))
        avp = es.enter_context(tc.tile_pool(name="av", bufs=1, space="PSUM"))
        psp = es.enter_context(tc.tile_pool(name="ps", bufs=2, space="PSUM"))

        ones_r = cpool.tile([1, 128], F32, name="tl", tag="ones_r")
        nc.gpsimd.memset(ones_r[:], 1.0)
        ones64 = cpool.tile([128, 64], F32, name="tl", tag="ones64")
        nc.gpsimd.memset(ones64[:], 1.0)
        ones64b = cpool.tile([128, 64], BF16, name="tl", tag="ones64b")
        nc.gpsimd.memset(ones64b[:], 1.0)
        eps_c = cpool.tile([1, 1], F32, name="tl", tag="eps_c")
        nc.gpsimd.memset(eps_c[:], 1e-5)

        # initial descriptor state: one DMA
        nc.sync.dma_start(dstb[:, :, :, :], xcb[:, :, :, :])

        qpads, qp_ctr = [], [0]
        if not SC_ROWSPLIT:
            for bi in range(2):
                qp = [cpool.tile([128, 256], BF16, name="tl", tag=f"qpad{bi}_{c}")
                      for c in range(2)]
                for c in range(2):
                    nc.gpsimd.memset(qp[c][:], 0.0)
                qpads.append(qp)

        def load_bias(src_ap, n, tag):
            t_ = bp.tile([128, n // 128], F32, name="tl", tag=tag)
            nc.sync.dma_start(t_[:], src_ap.rearrange("(a p) -> p a", p=128))
            return t_

        for i in [li for _r in range(reps) for li in range(n_layers)]:
            props = LAYER_PROPS[i]
            lns = LAYER_LNS[i]
            g0, g1 = grp_split(props)
            gidx = {}
            for g, idxs in enumerate([g0, g1]):
                for k, j in enumerate(idxs):
                    gidx[j] = (g, k)
            brs = sorted(set(p[0] for p in props))

            # ---- prefetch packed weights
            bpk, apk = {}, {}
            for br in brs:
                t = wbp.tile([128, 4096], BF16, name="tl", tag="bpk")
                nc.sync.dma_start(t[:], wpackB[i, br])
                bpk[br] = t
                t = wap.tile([128, 1024], BF16, name="tl", tag="apk")
                nc.sync.dma_start(t[:], wpackA[i, br])
                apk[br] = t

            if not zb:
                bk_t = {br: load_bias(pbk[i, br], 256, "bk") for br in brs}
                bq_t = {br: load_bias(pbq[i, br], 256, "bq") for br in brs}
                mb_t = {br: load_bias(mbb[i, br], 256, "mb") for br in brs}
                b1_t = {br: load_bias(b1b[i, br], 512, "b1") for br in brs}
                b2_t = {br: load_bias(b2b[i, br], 256, "b2") for br in brs}
                bv_b = {}
                for br in brs:
                    bvr = sp_.tile([1, 256], F32, name="tl", tag="bvr")
                    nc.sync.dma_start(bvr[:], pbv[i, br][None, :])
                    bv_ps = psp.tile([128, 256], F32, name="tl", tag="ps")
                    nc.tensor.matmul(bv_ps[:], ones_r[:], bvr[:], start=True, stop=True)
                    t = sp_.tile([128, 256], F32, name="tl", tag="bvb")
                    nc.vector.tensor_copy(t[:], bv_ps[:])
                    bv_b[br] = t

            # ---- stage A: local k/v chunks per pair, AllGather per group
            for g, idxs in enumerate([g0, g1]):
                if not idxs or "A" not in stages:
                    continue
                for k, j in enumerate(idxs):
                    br, xi, si = props[j]
                    a = apk[br]
                    kv_sb = kvwp.tile([128, 512], BF16, name="tl", tag="kvw")
                    for c in range(2):
                        kc_ps = psp.tile([128, 512], F32, name="tl", tag="ps")
                        for cc in range(2):
                            nc.tensor.matmul(
                                kc_ps[:, 0:CH],
                                a[:, cc * 256 + c * 128:cc * 256 + c * 128 + 128],
                                dstb[:, si, cc, :], start=(cc == 0), stop=(cc == 1))
                        if zb:
                            nc.vector.tensor_copy(kv_sb[:, c * 128:(c + 1) * 128],
                                                  kc_ps[:, 0:CH])
                        else:
                            nc.scalar.activation(kv_sb[:, c * 128:(c + 1) * 128],
                                                 kc_ps[:, 0:CH], AF.Identity,
                                                 bias=bk_t[br][:, c:c + 1])
                    vt_ps = psp.tile([128, 512], F32, name="tl", tag="ps")
                    for cc in range(2):
                        nc.tensor.matmul(vt_ps[:, 0:256], dstb[:, si, cc, :],
                                         a[:, 512 + cc * 256:512 + (cc + 1) * 256],
                                         start=(cc == 0), stop=(cc == 1))
                    if zb:
                        nc.vector.tensor_copy(kv_sb[:, 256:512], vt_ps[:, 0:256])
                    else:
                        nc.vector.tensor_add(kv_sb[:, 256:512], vt_ps[:, 0:256],
                                             bv_b[br][:])
                    nc.sync.dma_start(agin_t[i][g][k], kv_sb[:])
                nc.gpsimd.collective_compute(
                    "AllGather", mybir.AluOpType.bypass, replica_groups=rg,
                    ins=[agin_t[i][g].ap().opt()], outs=[agout_t[i][g].ap().opt()])

            # ---- stage B: per-prop attention + merge + MLP on local queries
            ndelta = {t: sum(1 for p in props if p[1] == t) for t in range(4)}
            seen = {t: 0 for t in range(4)}
            first_delta = {t: True for t in range(4)}
            ln_done = set()

            def emit_ln(t):
                # residual + LayerNorm over channel dim (partitions x 2 ctiles)
                if "C" not in stages:
                    return
                xq = lnp.tile([128, 1024], F32, name="tl", tag="xq")
                s_ps = psp.tile([128, 512], F32, name="tl", tag="ps")
                for c in range(2):
                    nc.vector.tensor_add(xq[:, c * 256:c * 256 + CH],
                                         dstb[:, t, c, :],
                                         dlt[:, t, c * CH:(c + 1) * CH])
                    nc.vector.tensor_mul(xq[:, c * 256 + CH:c * 256 + 2 * CH],
                                         xq[:, c * 256:c * 256 + CH],
                                         xq[:, c * 256:c * 256 + CH])
                    nc.tensor.matmul(s_ps[0:64, 0:256], ones64[:],
                                     xq[:, c * 256:(c + 1) * 256],
                                     start=(c == 0), stop=(c == 1))
                st = sp_.tile([1, 512], F32, name="tl", tag="st")
                nc.vector.tensor_scalar_mul(st[:, 0:CH], s_ps[0:1, 0:CH], 1.0 / 256)
                nc.vector.tensor_scalar_mul(st[:, CH:256], s_ps[0:1, CH:256], 1.0 / 256)
                nc.vector.tensor_mul(st[:, 256:384], st[:, 0:CH], st[:, 0:CH])
                nc.vector.tensor_sub(st[:, 256:384], st[:, CH:256], st[:, 256:384])
                # rs = (var+eps)^-0.5 via ln/exp (stays in the exp table set)
                nc.scalar.activation(st[:, 384:512], st[:, 256:384], AF.Ln,
                                     bias=eps_c[:])
                nc.scalar.activation(st[:, 256:384], st[:, 384:512], AF.Exp,
                                     scale=-0.5)
                bc_ps = psp.tile([128, 512], F32, name="tl", tag="ps")
                nc.tensor.matmul(bc_ps[:, 0:CH], ones_r[:], st[:, 0:CH],
                                 start=True, stop=True)
                nc.tensor.matmul(bc_ps[:, CH:256], ones_r[:], st[:, 256:384],
                                 start=True, stop=True)
                if not ln_triv:
                    g_row = sp_.tile([1, 256], F32, name="tl", tag="grow")
                    nc.sync.dma_start(g_row[:], lng[i, t][None, :])
                    b_col = bp.tile([128, 2], F32, name="tl", tag="lnb")
                    nc.sync.dma_start(b_col[:], lnb[i, t].rearrange("(a p) -> p a", p=128))
                bc_sb = abp.tile([128, 256], F32, name="tl", tag="bcln")
                nc.vector.tensor_copy(bc_sb[:], bc_ps[:, 0:256])
                for c in range(2):
                    tmp = tmpp.tile([128, CH], F32, name="tl", tag="tmp")
                    nc.vector.tensor_sub(tmp[:], xq[:, c * 256:c * 256 + CH],
                                         bc_sb[:, 0:CH])
                    if ln_triv:
                        nc.vector.tensor_mul(dstb[:, t, c, :], tmp[:],
                                             bc_sb[:, CH:256])
                    else:
                        g_ps = psp.tile([128, 512], F32, name="tl", tag="ps")
                        nc.tensor.matmul(g_ps[:, 0:CH],
                                         g_row[:, c * 128:(c + 1) * 128],
                                         st[:, 256:384], start=True, stop=True)
                        gs = tmpp.tile([128, CH], F32, name="tl", tag="gs")
                        nc.vector.tensor_copy(gs[:], g_ps[:, 0:CH])
                        nc.vector.tensor_mul(tmp[:], tmp[:], gs[:])
                        nc.vector.tensor_scalar_add(dstb[:, t, c, :], tmp[:],
                                                    b_col[:, c:c + 1])

            for j in (g0 + g1 if "B" in stages else []):
                br, xi, si = props[j]
                g, k = gidx[j]
                b = bpk[br]
                kvg = kvgp.tile([128, NC * 512], BF16, name="tl", tag="kvg")
                nc.sync.dma_start(
                    kvg[:].rearrange("p (r f) -> p r f", r=NC),
                    agout_t[i][g].ap().rearrange("r q p f -> q p r f")[k])

                # q projection -> per-ctile head-stacked tiles (bf16)
                qh = [qhp.tile([128, CH], BF16, name="tl", tag="qh") for _ in range(2)]
                if not SC_ROWSPLIT:
                    qh = qpads[qp_ctr[0] % 2]
                    qp_ctr[0] += 1
                for c in range(2):
                    q_ps = psp.tile([128, 512], F32, name="tl", tag="ps")
                    for cc in range(2):
                        nc.tensor.matmul(
                            q_ps[:, 0:CH],
                            b[:, cc * 256 + c * 128:cc * 256 + c * 128 + 128],
                            dstb[:, xi, cc, :], start=(cc == 0), stop=(cc == 1))
                    if not SC_ROWSPLIT:
                        nc.vector.tensor_copy(qh[c][0:64, 0:CH], q_ps[0:64, 0:CH])
                        nc.vector.tensor_copy(qh[c][64:128, CH:2 * CH],
                                              q_ps[64:128, 0:CH])
                    elif zb:
                        nc.vector.tensor_copy(qh[c][:], q_ps[:, 0:CH])
                    else:
                        nc.scalar.activation(qh[c][:], q_ps[:, 0:CH], AF.Identity,
                                             bias=bq_t[br][:, c:c + 1])

                if blvl < 2:
                    continue
                # scores^T (keys x [h,q]) via two concurrent K=64 row-tiles; exp
                e_t = []
                for mp in range(NC // 2):
                    sc_ps = scp.tile([128, 1024], F32, name="tl", tag="sc")
                    for ml in range(2):
                        m = 2 * mp + ml
                        for c in range(2):
                            if not SC_ROWSPLIT:
                                nc.tensor.matmul(
                                    sc_ps[:, ml * 512 + (2 * c) * CH:
                                          ml * 512 + (2 * c + 2) * CH],
                                    kvg[:, m * 512 + c * 128:m * 512 + (c + 1) * 128],
                                    qh[c][:], start=True, stop=True)
                                continue
                            for hh in range(2):
                                nc.tensor.matmul(
                                    sc_ps[:, ml * 512 + (2 * c + hh) * CH:
                                          ml * 512 + (2 * c + hh + 1) * CH],
                                    kvg[64 * hh:64 * hh + 64,
                                        m * 512 + c * 128:m * 512 + (c + 1) * 128],
                                    qh[c][64 * hh:64 * hh + 64, :],
                                    start=True, stop=True,
                                    tile_position=(64 * hh, 0))
                    e_sb = ep.tile([128, 1024], BF16, name="tl", tag="exps")
                    if EXP_SPLIT:
                        nc.scalar.activation(e_sb[:, 0:512], sc_ps[:, 0:512], AF.Exp)
                        nc.scalar.activation(e_sb[:, 512:1024], sc_ps[:, 512:1024],
                                             AF.Exp)
                    else:
                        nc.scalar.activation(e_sb[:], sc_ps[:], AF.Exp)
                    e_t.append(e_sb)

                # Z accumulation (DVE, bf16)
                if blvl >= 3:
                    z_acc = zap.tile([128, 512], BF16, name="tl", tag="zacc")
                    nc.vector.tensor_copy(z_acc[:], e_t[0][:, 0:512])
                    for m in range(1, NC):
                        nc.vector.tensor_add(
                            z_acc[:], z_acc[:],
                            e_t[m // 2][:, (m % 2) * 512:(m % 2) * 512 + 512])

                # A @ V accumulated over m-tiles (one PSUM bank per ctile)
                av_ps = [avp.tile([128, CH], F32, name="tl", tag=f"av{c}")
                         for c in range(2)]
                for m in (range(NC) if blvl >= 4 else []):
                    e_sl = e_t[m // 2]
                    off = (m % 2) * 512
                    for h in range(H):
                        c, o = h // 2, 64 * (h % 2)
                        nc.tensor.matmul(
                            av_ps[c][o:o + 64, :],
                            kvg[:, m * 512 + 256 + 64 * h:m * 512 + 256 + 64 * h + 64],
                            e_sl[:, off + h * CH:off + (h + 1) * CH],
                            start=(m == 0), stop=(m == NC - 1),
                            tile_position=(0, o), skip_group_check=True)

                if blvl < 5:
                    continue
                # normalize: attn = av * (1/Z) broadcast
                z_ps = psp.tile([128, 512], F32, name="tl", tag="ps")
                nc.tensor.matmul(z_ps[0:64, :], ones64b[:], z_acc[:],
                                 start=True, stop=True)
                r_row = sp_.tile([1, 512], F32, name="tl", tag="rz")
                nc.vector.reciprocal(r_row[:], z_ps[0:1, :])
                bc_ps = psp.tile([128, 512], F32, name="tl", tag="ps")
                for h in range(H):
                    c, o = h // 2, 64 * (h % 2)
                    nc.tensor.matmul(bc_ps[o:o + 64, c * CH:(c + 1) * CH],
                                     ones_r[:, 0:64], r_row[:, h * CH:(h + 1) * CH],
                                     start=True, stop=True, tile_position=(0, o))
                bc_sb = abp.tile([128, 256], F32, name="tl", tag="bcav")
                nc.vector.tensor_copy(bc_sb[:], bc_ps[:, 0:256])
                attn_sb = atp.tile([128, 256], BF16, name="tl", tag="attn")
                for c in range(2):
                    nc.vector.tensor_mul(attn_sb[:, c * CH:(c + 1) * CH],
                                         av_ps[c][:], bc_sb[:, c * CH:(c + 1) * CH])

                if blvl < 6:
                    continue
                # merge
                m_ps = psp.tile([128, 512], F32, name="tl", tag="ps")
                for c in range(2):
                    for cc in range(2):
                        nc.tensor.matmul(
                            m_ps[:, c * CH:(c + 1) * CH],
                            b[:, 512 + cc * 256 + c * 128:512 + cc * 256 + c * 128 + 128],
                            attn_sb[:, cc * CH:(cc + 1) * CH],
                            start=(cc == 0), stop=(cc == 1))
                msg_sb = msp.tile([128, 256], BF16, name="tl", tag="msg")
                if zb:
                    nc.vector.tensor_copy(msg_sb[:], m_ps[:, 0:256])
                else:
                    for c in range(2):
                        nc.scalar.activation(msg_sb[:, c * CH:(c + 1) * CH],
                                             m_ps[:, c * CH:(c + 1) * CH],
                                             AF.Identity, bias=mb_t[br][:, c:c + 1])

                if blvl < 7:
                    continue
                # mlp1 (relu) on concat([x, msg]); two c's share one PSUM bank
                def h_in(cc):
                    if cc < 2:
                        return dstb[:, xi, cc, :]
                    return msg_sb[:, (cc - 2) * CH:(cc - 1) * CH]
                h1_sb = []
                for pair in range(2):
                    h_ps = psp.tile([128, 512], F32, name="tl", tag="ps")
                    for cl in range(2):
                        c = pair * 2 + cl
                        for cc in range(4):
                            nc.tensor.matmul(
                                h_ps[:, cl * CH:(cl + 1) * CH],
                                b[:, 1024 + cc * 512 + c * 128:
                                  1024 + cc * 512 + c * 128 + 128],
                                h_in(cc), start=(cc == 0), stop=(cc == 3))
                    t_ = h1p.tile([128, 256], BF16, name="tl", tag="h1")
                    if zb:
                        nc.vector.tensor_relu(t_[:], h_ps[:, 0:256])
                    else:
                        for cl in range(2):
                            c = pair * 2 + cl
                            nc.scalar.activation(t_[:, cl * CH:(cl + 1) * CH],
                                                 h_ps[:, cl * CH:(cl + 1) * CH],
                                                 AF.Relu, bias=b1_t[br][:, c:c + 1])
                    h1_sb.append(t_)

                if blvl < 8:
                    continue
                # mlp2 -> delta accumulation
                d_ps = psp.tile([128, 512], F32, name="tl", tag="ps")
                for c in range(2):
                    for cc in range(4):
                        nc.tensor.matmul(
                            d_ps[:, c * CH:(c + 1) * CH],
                            b[:, 3072 + cc * 256 + c * 128:
                              3072 + cc * 256 + c * 128 + 128],
                            h1_sb[cc // 2][:, (cc % 2) * CH:(cc % 2 + 1) * CH],
                            start=(cc == 0), stop=(cc == 3))
                if zb:
                    if first_delta[xi]:
                        nc.vector.tensor_copy(dlt[:, xi, :], d_ps[:, 0:256])
                    else:
                        nc.vector.tensor_add(dlt[:, xi, :], dlt[:, xi, :],
                                             d_ps[:, 0:256])
                else:
                    tmp = tmpp.tile([128, 256], F32, name="tl", tag="dtmp")
                    for c in range(2):
                        nc.scalar.activation(tmp[:, c * CH:(c + 1) * CH],
                                             d_ps[:, c * CH:(c + 1) * CH],
                                             AF.Identity, bias=b2_t[br][:, c:c + 1])
                    if first_delta[xi]:
                        nc.vector.tensor_copy(dlt[:, xi, :], tmp[:])
                    else:
                        nc.vector.tensor_add(dlt[:, xi, :], dlt[:, xi, :], tmp[:])
                first_delta[xi] = False
                seen[xi] += 1
                if seen[xi] == ndelta[xi] and xi in lns and xi not in ln_done:
                    ln_done.add(xi)
                    emit_ln(xi)

            if "C" in stages and "B" not in stages:
                for t in lns:
                    emit_ln(t)

        # ---- epilogue: out[m] = (1/32) qvec^T kmat[:, m]
        s1 = sp_.tile([128, 2], F32, name="tl", tag="s1")
        for c in range(2):
            nc.vector.reduce_sum(s1[:, c:c + 1], dstb[:, 1, c, :],
                                 axis=mybir.AxisListType.X)
            nc.sync.dma_start(ag2in[c], s1[:, c:c + 1])
        nc.gpsimd.collective_compute(
            "AllGather", mybir.AluOpType.bypass, replica_groups=rg,
            ins=[ag2in.ap().opt()], outs=[ag2out.ap().opt()])
        gath = sp_.tile([128, NC], F32, name="tl", tag="gath")
        d1b = sp_.tile([128, 2], F32, name="tl", tag="d1b")
        d1bb = sp_.tile([128, 2], BF16, name="tl", tag="d1bb")
        for c in range(2):
            nc.sync.dma_start(gath[:], ag2out.ap().rearrange("r c p o -> c p (r o)")[c])
            nc.vector.reduce_sum(d1b[:, c:c + 1], gath[:], axis=mybir.AxisListType.X)
        nc.vector.tensor_copy(d1bb[:], d1b[:])

        wq5 = [cpool.tile([128, 256], BF16, name="tl", tag=f"wq5{k}") for k in range(2)]
        wk5 = [cpool.tile([128, 256], BF16, name="tl", tag=f"wk5{k}") for k in range(2)]
        for k in range(2):
            nc.sync.dma_start(wq5[k][:], w5T[0, k * 128:(k + 1) * 128, :])
            nc.sync.dma_start(wk5[k][:], w5T[1, k * 128:(k + 1) * 128, :])
        b5 = bp.tile([128, 4], F32, name="tl", tag="b5")
        nc.sync.dma_start(b5[:], pb5.rearrange("t (a p) -> p (t a)", p=128))
        qv = sp_.tile([128, 2], F32, name="tl", tag="qv")
        for c in range(2):
            q_ps = psp.tile([128, 512], F32, name="tl", tag="ps")
            for cc in range(2):
                nc.tensor.matmul(q_ps[:, 0:1], wq5[cc][:, c * 128:(c + 1) * 128],
                                 d1bb[:, cc:cc + 1], start=(cc == 0), stop=(cc == 1))
            nc.scalar.activation(qv[:, c:c + 1], q_ps[:, 0:1], AF.Identity,
                                 bias=b5[:, c:c + 1], scale=1.0 / N)
        km = [sp_.tile([128, CH], F32, name="tl", tag=f"km{c}") for c in range(2)]
        for c in range(2):
            k_ps = psp.tile([128, 512], F32, name="tl", tag="ps")
            for cc in range(2):
                nc.tensor.matmul(k_ps[:, 0:CH], wk5[cc][:, c * 128:(c + 1) * 128],
                                 dstb[:, 0, cc, :], start=(cc == 0), stop=(cc == 1))
            nc.scalar.activation(km[c][:], k_ps[:, 0:CH], AF.Identity,
                                 bias=b5[:, 2 + c:3 + c])
        o_ps = psp.tile([128, 512], F32, name="tl", tag="ps")
        for c in range(2):
            nc.vector.tensor_scalar_mul(km[c][:], km[c][:], qv[:, c:c + 1])
            nc.tensor.matmul(o_ps[0:64, 0:CH], ones64[:], km[c][:],
                             start=(c == 0), stop=(c == 1))
        o_sb = sp_.tile([1, CH], F32, name="tl", tag="osb")
        nc.scalar.activation(o_sb[:], o_ps[0:1, 0:CH], AF.Copy, scale=1.0 / 32)
        nc.sync.dma_start(out_d[:], o_sb[:])

    nc.compile()
    return nc


def prep_inputs(inputs, zb=True, ln_triv=True):
    inp = {k: np.ascontiguousarray(np.asarray(v)) for k, v in inputs.items()}
    pw, pb = inp['proj_w'].astype(np.float32), inp['proj_b'].astype(np.float32)
    mw = inp['merge_w'].astype(np.float32)
    w1 = inp['mlp_w1'].astype(np.float32)
    w2 = inp['mlp_w2'].astype(np.float32)

    bf = mybir.dt.np(mybir.dt.bfloat16)
    wpackA = np.empty((5, 3, 128, 1024), np.float32)
    wpackB = np.empty((5, 3, 128, 4096), np.float32)
    pbq = np.empty((5, 3, 256), np.float32)
    pbk = np.empty((5, 3, 256), np.float32)
    pbv = np.empty((5, 3, 256), np.float32)
    for i in range(5):
        for br in range(3):
            wqT = pw[br, i, 0][PERM].T * 0.125   # [256 in, 256 out]
            wkT = pw[br, i, 1][PERM].T
            wvT = pw[br, i, 2][PERM].T
            mgT = mw[br, i][:, PERM].T
            w1T = w1[br, i].T                     # [512, 512]
            w2T = w2[br, i].T                     # [512, 256]
            wpackA[i, br, :, 0:256] = wkT[0:128]
            wpackA[i, br, :, 256:512] = wkT[128:256]
            wpackA[i, br, :, 512:768] = wvT[0:128]
            wpackA[i, br, :, 768:1024] = wvT[128:256]
            wpackB[i, br, :, 0:256] = wqT[0:128]
            wpackB[i, br, :, 256:512] = wqT[128:256]
            wpackB[i, br, :, 512:768] = mgT[0:128]
            wpackB[i, br, :, 768:1024] = mgT[128:256]
            for cc in range(4):
                wpackB[i, br, :, 1024 + cc * 512:1024 + (cc + 1) * 512] = \
                    w1T[cc * 128:(cc + 1) * 128]
                wpackB[i, br, :, 3072 + cc * 256:3072 + (cc + 1) * 256] = \
                    w2T[cc * 128:(cc + 1) * 128]
            pbq[i, br] = pb[br, i, 0][PERM] * 0.125
            pbk[i, br] = pb[br, i, 1][PERM]
            pbv[i, br] = pb[br, i, 2][PERM]

    w5T = np.stack([pw[0, 5, 0].T, pw[0, 5, 1].T]).astype(bf)
    pb5 = np.stack([pb[0, 5, 0], pb[0, 5, 1]]).astype(np.float32)

    desc = np.stack([inp[f'desc{t}'][0] for t in range(4)]).astype(np.float32)
    shared = dict(wpackA=wpackA.astype(bf), wpackB=wpackB.astype(bf),
                  w5T=w5T, pb5=pb5)
    if not zb:
        mb = inp['merge_b'].astype(np.float32)
        b1 = inp['mlp_b1'].astype(np.float32)
        b2 = inp['mlp_b2'].astype(np.float32)
        shared.update(
            pbq=pbq, pbk=pbk, pbv=pbv,
            mbb=np.ascontiguousarray(np.transpose(mb[:, :5], (1, 0, 2))),
            b1b=np.ascontiguousarray(np.transpose(b1[:, :5], (1, 0, 2))),
            b2b=np.ascontiguousarray(np.transpose(b2[:, :5], (1, 0, 2))))
    if not ln_triv:
        ng = inp['norm_g'].astype(np.float32)
        nb = inp['norm_b'].astype(np.float32)
        shared.update(
            lng=np.ascontiguousarray(np.transpose(ng[:, :5], (1, 0, 2))),
            lnb=np.ascontiguousarray(np.transpose(nb[:, :5], (1, 0, 2))))

    in_maps = []
    for j in range(NC):
        xcj = desc[:, :, j * CH:(j + 1) * CH].reshape(4, 2, 128, CH)
        xcb = np.ascontiguousarray(np.transpose(xcj, (2, 0, 1, 3))).astype(bf)
        in_maps.append({"xcb": xcb, **shared})
    return in_maps


def kernel(**inputs):
    zb = all(not np.asarray(inputs[k]).any() for k in
             ("proj_b", "merge_b", "mlp_b1", "mlp_b2"))
    ln_triv = (np.asarray(inputs["norm_g"])[:, :5] == 1).all() and \
        not np.asarray(inputs["norm_b"])[:, :5].any()
    key = f"nc{zb}_{ln_triv}"
    if key not in _cache:
        _cache[key] = build_kernel(zb=zb, ln_triv=ln_triv)
    nc = _cache[key]
    in_maps = prep_inputs(inputs, zb=zb, ln_triv=ln_triv)
    res = run_bass_kernel_spmd(nc, in_maps, core_ids=list(range(NC)))
    out = np.concatenate([res.results[j]["out"][0] for j in range(NC)])
    mask = np.asarray(inputs["unreachable"]).any(axis=0)
    out = np.where(mask, np.float32(-1e9), out.astype(np.float32))
    return out
